# revision 22
# baseline (speedup 1.0000x reference)
"""BlendShapes model kernel for 8 Trainium2 NeuronCores (warm-PE design).

Computation (reference):
    pose_repr = pose[:, 1:].reshape(B, 23, 9) - eye      # (B, J, 9)
    per-joint MLP 9 -> 18 -> 32 -> 8 (ReLU between)      # coff (B, J, 8)
    basis_full = basis[:, None] * mask[:, :, None, None]  # (V, J, 8, 3)
    res = einsum('bjk,vjkc->bvc', coff, basis_full)       # (B, V, 3)

Mapping (per core; vertices sharded 8 ways, V=6890 padded to 8*864):
  - Host precomputes bfm = basis*mask*2^13 (f16, rows (j,k), cols (v,c)).
    ALL biases are folded into matmuls so every PSUM evacuation is a pure
    ReLU / scaled-copy (runs on either ACT or DVE, no bias operand):
      * eye-subtraction -> L1 bias (b1' = b1 - e @ W1)
      * b1', b2 -> ones-row trick (activations carry a constant-1 row,
        weights carry the bias as an extra contraction row)
      * b3 -> folded into the main GEMM: bias_vc = b3 . bfm is a
        b-independent column vector, added via a ones-row in the K=57
        B-pass (coffT_b row 56 = 1, bfm_b row 56 = bias_vc).
  - The PE's HAM clock gate throttles matmuls to 1.2 GHz until ~6us of
    sustained activity, then 2.4 GHz. Warm-up matmuls run during the input
    DMAs; "pre-matmuls" into each upcoming PSUM tile keep the PE dense
    through the MLP's epilogue-paced stretches.
  - Input DMAs: one mega DMA (w + pose images) on the sync queue; bfm on
    the gpsimd queue fenced behind the mega DMA so its 1.3MB doesn't starve
    the MLP critical path (DMA engines arbitrate between queues in bursts).
  - MLP joint chunks of 4 (6 chunks):
      L1 (K=37, M=72):  chunk pairs row-tiled at PE rows 0 / 64 -> 2x
      L2 (K=73, M=128): plain matmuls
      L3 (K=128, M=32): col-tiled 4-way straight into coffT layout
  - Main GEMM out[b, (v,c)] = coffT.T @ bfm, K=184 split 128+56(+bias row),
    b-tiles in pairs: A-passes, then both K=57 B-passes CONCURRENTLY in PE
    row groups 0 / 64; per-chunk evacuation on ACT (i) and DVE (j).
  - Output stored f16 (descale 2^-13 in the evacuation); host converts.
"""

import numpy as np

N_VERT, N_JOINT, BPJ, BATCH = 6890, 23, 8, 1024
VPAD = 6912  # 8 * 864
VC = VPAD // 8  # 864 vertices per core
VC3 = VC * 3  # 2592
NB = BATCH // 128  # 8 b-tiles
NT_BOUNDS = [0, 512, 1024, 1536, 2048, 2560, 2592]
PAIR_BOUNDS = [0, 1024, 2048, 2592]

CHUNKS = [(0, 4), (4, 8), (8, 12), (12, 16), (16, 20), (20, 23)]


def _offsets(mpj):
    offs, col = [], 0
    for js, je in CHUNKS:
        offs.append(col)
        col += (je - js) * mpj
    return offs, col


W1_OFF, W1_TOT = _offsets(18)  # 414
W2_OFF, W2_TOT = _offsets(32)  # 736
W3_OFF, W3_TOT = _offsets(8)   # 184
W2_OFF = [W1_TOT + o for o in W2_OFF]
W3_OFF = [W1_TOT + W2_TOT + o for o in W3_OFF]
W_COLS = W1_TOT + W2_TOT + W3_TOT  # 1334

BSCALE = 8192.0  # 2**13, exact in f16/f32
DESCALE = 1.0 / 8192.0
N_WARMUP = 11  # warm-up matmuls (N=512) before the MLP

_CACHED = {}


def _build_nc():
    import concourse.tile as tile
    from concourse import bacc, mybir
    from contextlib import ExitStack

    dt = mybir.dt
    f32, f16 = dt.float32, dt.float16
    AF = mybir.ActivationFunctionType
    ALU = mybir.AluOpType

    nc = bacc.Bacc(None, target_bir_lowering=False)

    # mega input: [128, W_COLS + 3*1024] f16 = w_all columns followed by the
    # three pose-pair tile images (chunk 2p at rows 0.., 2p+1 at rows 64..,
    # each with its constant-1 bias row baked in).
    MEGA_COLS = W_COLS + 3 * BATCH
    mega_d = nc.dram_tensor("mega", [128, MEGA_COLS], f16, kind="ExternalInput")
    bfm_a_d = nc.dram_tensor("bfm_a", [128, VC3], f16, kind="ExternalInput")
    # bfm_b rows: 0-55 data, 56 bias_vc, 64-119 data dup, 120 bias_vc dup
    # (the B-pass runs two b-tiles concurrently in PE row groups 0 and 64).
    bfm_b_d = nc.dram_tensor("bfm_b", [128, VC3], f16, kind="ExternalInput")
    ones_d = nc.dram_tensor("ones", [1, 6 * BATCH], f16, kind="ExternalInput")
    res = nc.dram_tensor("res", [BATCH, VC3], f16, kind="ExternalOutput")

    with ExitStack() as ctx:
        tc = ctx.enter_context(tile.TileContext(nc))
        const = ctx.enter_context(tc.tile_pool(name="const", bufs=1))
        work = ctx.enter_context(tc.tile_pool(name="work", bufs=1))
        outp = ctx.enter_context(tc.tile_pool(name="outp", bufs=4))
        psum = ctx.enter_context(tc.tile_pool(name="psum", bufs=4, space="PSUM"))

        # warm-up source: memset on DVE (its queue opens early); a tiny
        # ACTIVATE right away pulls the 1.3us ACT table load off the
        # critical path.
        warm = work.tile([128, 512], f16, tag="warm")
        nc.vector.memset(warm[:], 0.0)
        actwarm = work.tile([128, 16], f16, tag="actwarm")
        nc.scalar.activation(actwarm[0:1, :], warm[0:1, 0:16], AF.Relu, bias=0.0)

        # ---- input DMAs.
        mega = const.tile([128, MEGA_COLS], f16, tag="mega")
        nc.sync.dma_start(out=mega[:], in_=mega_d[:, :])
        w_sb = mega[:, 0:W_COLS]
        pose_p = [
            mega[:, W_COLS + p * BATCH : W_COLS + (p + 1) * BATCH] for p in range(3)
        ]

        # h1_all: columns 1024c hold chunk c's activations; row 72 (row 54
        # for chunk 5) carries the constant-1 for the L2 ones-row bias.
        h1_all = work.tile([128, 6 * BATCH], f16, tag="h1_all")
        coffT_a = work.tile([128, BATCH], f16, tag="coffT_a")
        coffT_b = work.tile([128, BATCH], f16, tag="coffT_b")
        nc.gpsimd.dma_start(out=h1_all[72:73, :], in_=ones_d[0:1, :])
        nc.gpsimd.dma_start(
            out=h1_all[54:55, 5 * BATCH : 6 * BATCH], in_=ones_d[0:1, 0:BATCH]
        )
        nc.gpsimd.dma_start(out=coffT_b[56:57, :], in_=ones_d[0:1, 0:BATCH])
        nc.gpsimd.dma_start(out=coffT_b[120:121, :], in_=ones_d[0:1, 0:BATCH])

        # fence: a byte in each bfm tile that depends on the mega data, so
        # the bfm DMAs (WAW) can't start until the mega DMA completed.
        bfm_a = work.tile([128, VC3], f16, tag="bfm_a")
        bfm_b = work.tile([128, VC3], f16, tag="bfm_b")
        nc.gpsimd.tensor_scalar(
            out=bfm_a[64:65, 0:1], in0=pose_p[2][64:65, 1023:1024], scalar1=1.0,
            scalar2=None, op0=ALU.mult,
        )
        nc.gpsimd.tensor_scalar(
            out=bfm_b[32:33, 0:1], in0=pose_p[2][64:65, 1022:1023], scalar1=1.0,
            scalar2=None, op0=ALU.mult,
        )
        nc.gpsimd.dma_start(out=bfm_a[:], in_=bfm_a_d[:, :])
        nc.gpsimd.dma_start(out=bfm_b[:], in_=bfm_b_d[:, :])

        # ---- PSUM allocation with a reserved filler buffer: the pool
        # rotates 4 bufs; whenever the rotation would hand buf0 to a real
        # tile, a dummy "fill" tile takes that slot instead. Filler matmuls
        # target the fill tile, so they NEVER wait on real-tile evacuations
        # and can keep the PE's HAM activity window busy during the MLP's
        # epilogue-paced stretches. The main loop allocates its tiles
        # through ps_tile too but emits no fillers (its stream is dense).
        alloc_ctr = [0]
        fill = [None]

        def ps_tile(name, reserve=True):
            if reserve and alloc_ctr[0] % 4 == 0:
                fill[0] = psum.tile(
                    [128, 1024], f32, tag="ps", name=f"fill_{alloc_ctr[0]}"
                )
                alloc_ctr[0] += 1
            t = psum.tile([128, 1024], f32, tag="ps", name=name)
            alloc_ctr[0] += 1
            return t

        def filler(n=1):
            for _ in range(n):
                nc.tensor.matmul(
                    fill[0][:, 0:512], lhsT=warm[:, 0:128], rhs=warm[:],
                    start=True, stop=True, skip_group_check=True,
                )

        # ---- PE warm-up.
        fill[0] = psum.tile([128, 1024], f32, tag="ps", name="warm_ps")
        alloc_ctr[0] = 1
        filler(N_WARMUP)

        ep_ctr = [0]

        def epilogue(dst, src, relu=False, scale=None):
            # pure ReLU / copy -- either engine; alternate for balance.
            e = ep_ctr[0] % 5
            ep_ctr[0] += 1
            if e < 3:  # ACT is ~10% faster per op; give it 3/5 of the load
                if relu:
                    nc.scalar.activation(dst, src, AF.Relu, bias=0.0)
                else:
                    nc.scalar.activation(
                        dst, src, AF.Copy, scale=1.0 if scale is None else scale
                    )
            elif relu:
                nc.vector.tensor_scalar(
                    out=dst, in0=src, scalar1=0.0, scalar2=None, op0=ALU.max
                )
            else:
                nc.vector.tensor_scalar(
                    out=dst, in0=src, scalar1=1.0 if scale is None else scale,
                    scalar2=None, op0=ALU.mult,
                )

        h2 = {}
        HALVES = (slice(0, 512), slice(512, 1024))

        def KM1(c):
            nj = CHUNKS[c][1] - CHUNKS[c][0]
            return 9 * nj + 1, 18 * nj  # +1: ones/bias row

        # L1: row-tiled chunk pairs (rows 0 / 64), both halves of B, one
        # [*,1024] PSUM tile per chunk -> one pure-ReLU epilogue per chunk.
        for p in range(3):
            c0, c1 = 2 * p, 2 * p + 1
            K0, M0 = KM1(c0)
            K1, M1 = KM1(c1)
            ps0 = ps_tile(f"ps1_{c0}")
            ps1 = ps_tile(f"ps1_{c1}")
            for h, hs in enumerate(HALVES):
                nc.tensor.matmul(
                    ps0[0:M0, hs], lhsT=w_sb[0:K0, W1_OFF[c0] : W1_OFF[c0] + M0],
                    rhs=pose_p[p][0:K0, hs], start=True, stop=True,
                    tile_position=(0, 0),
                )
                nc.tensor.matmul(
                    ps1[0:M1, hs], lhsT=w_sb[64 : 64 + K1, W1_OFF[c1] : W1_OFF[c1] + M1],
                    rhs=pose_p[p][64 : 64 + K1, hs], start=True, stop=True,
                    tile_position=(64, 0),
                )
            epilogue(h1_all[0:M0, c0 * BATCH : (c0 + 1) * BATCH], ps0[0:M0, :], relu=True)
            epilogue(h1_all[0:M1, c1 * BATCH : (c1 + 1) * BATCH], ps1[0:M1, :], relu=True)
            filler(2)

        # L2: plain per-chunk matmuls (K includes the ones/bias row).
        for c, (js, je) in enumerate(CHUNKS):
            nj = je - js
            K, M = 18 * nj + 1, 32 * nj
            h2[c] = work.tile([M, BATCH], f16, tag=f"h2_{c}", name=f"h2_{c}")
            ps = ps_tile(f"ps2_{c}")
            for h, hs in enumerate(HALVES):
                nc.tensor.matmul(
                    ps[0:M, hs], lhsT=w_sb[0:K, W2_OFF[c] : W2_OFF[c] + M],
                    rhs=h1_all[0:K, c * BATCH + hs.start : c * BATCH + hs.stop],
                    start=True, stop=True,
                )
            epilogue(h2[c][:, :], ps[0:M, :], relu=True)
            filler(2)

        # L3: col-tiled into coffT layout; pure-copy epilogues (b3 is folded
        # into the main B-pass via bias_vc). Group B lands twice (cols 0/32
        # and 64/96) for the concurrent B-passes.
        psA = ps_tile("ps3a")
        for h, hs in enumerate(HALVES):
            for c in range(4):
                nc.tensor.matmul(
                    psA[32 * c : 32 * c + 32, hs],
                    lhsT=w_sb[0:128, W3_OFF[c] : W3_OFF[c] + 32],
                    rhs=h2[c][:, hs], start=True, stop=True,
                    tile_position=(0, 32 * c),
                )
        epilogue(coffT_a[:, :], psA[:, :])
        filler(3)
        psB = ps_tile("ps3b")
        for h, hs in enumerate(HALVES):
            for r in (0, 64):
                nc.tensor.matmul(
                    psB[r : r + 32, hs], lhsT=w_sb[0:128, W3_OFF[4] : W3_OFF[4] + 32],
                    rhs=h2[4][:, hs], start=True, stop=True, tile_position=(0, r),
                )
                nc.tensor.matmul(
                    psB[r + 32 : r + 56, hs], lhsT=w_sb[0:96, W3_OFF[5] : W3_OFF[5] + 24],
                    rhs=h2[5][:, hs], start=True, stop=True, tile_position=(0, r + 32),
                )
        epilogue(coffT_b[0:56, :], psB[0:56, :])
        epilogue(coffT_b[64:120, :], psB[64:120, :])
        filler(4)

        # ---- main GEMM, b-tiles in pairs: per 1024-wide N-chunk, A-passes
        # (K=128) for both b-tiles, then the two K=57 B-passes (bias row
        # included) CONCURRENTLY in PE row groups 0 / 64; evacuation with
        # the 2^-13 descale on ACT (tile i) and DVE (tile j) in parallel.
        for p in range(NB // 2):
            bti, btj = 2 * p, 2 * p + 1
            bsl_i = slice(bti * 128, bti * 128 + 128)
            bsl_j = slice(btj * 128, btj * 128 + 128)
            os_i = outp.tile([128, VC3], f16, tag="ostrip", name=f"ostrip_{bti}")
            os_j = outp.tile([128, VC3], f16, tag="ostrip", name=f"ostrip_{btj}")
            for g in range(3):
                g0, g1 = PAIR_BOUNDS[g], PAIR_BOUNDS[g + 1]
                ti = ps_tile(f"psm_{p}_{g}_i", reserve=False)
                tj = ps_tile(f"psm_{p}_{g}_j", reserve=False)
                subs = [
                    (slice(n0 - g0, n1 - g0), slice(n0, n1))
                    for n0, n1 in zip(NT_BOUNDS, NT_BOUNDS[1:])
                    if g0 <= n0 < g1
                ]
                for ps, bsl in ((ti, bsl_i), (tj, bsl_j)):
                    for ssl, nsl in subs:
                        nc.tensor.matmul(
                            ps[:, ssl], lhsT=coffT_a[:, bsl], rhs=bfm_a[:, nsl],
                            start=True, stop=False,
                        )
                for ssl, nsl in subs:
                    nc.tensor.matmul(
                        ti[:, ssl], lhsT=coffT_b[0:57, bsl_i],
                        rhs=bfm_b[0:57, nsl], start=False, stop=True,
                        tile_position=(0, 0),
                    )
                    nc.tensor.matmul(
                        tj[:, ssl], lhsT=coffT_b[64:121, bsl_j],
                        rhs=bfm_b[64:121, nsl], start=False, stop=True,
                        tile_position=(64, 0),
                    )
                last = p == NB // 2 - 1
                if last and g == 2:
                    # final chunk: evacuate + store 512 then 32 cols so the
                    # very last transfer is tiny (short tail)
                    nc.scalar.activation(
                        os_i[:, 2048:2560], ti[:, 0:512], AF.Copy, scale=DESCALE
                    )
                    nc.vector.tensor_scalar(
                        out=os_j[:, 2048:2560], in0=tj[:, 0:512], scalar1=DESCALE,
                        scalar2=None, op0=ALU.mult,
                    )
                    nc.sync.dma_start(out=res[bsl_i, 2048:2560], in_=os_i[:, 2048:2560])
                    nc.gpsimd.dma_start(out=res[bsl_j, 2048:2560], in_=os_j[:, 2048:2560])
                    nc.scalar.activation(
                        os_i[:, 2560:2592], ti[:, 512:544], AF.Copy, scale=DESCALE
                    )
                    nc.vector.tensor_scalar(
                        out=os_j[:, 2560:2592], in0=tj[:, 512:544], scalar1=DESCALE,
                        scalar2=None, op0=ALU.mult,
                    )
                    nc.sync.dma_start(out=res[bsl_i, 2560:2592], in_=os_i[:, 2560:2592])
                    nc.gpsimd.dma_start(out=res[bsl_j, 2560:2592], in_=os_j[:, 2560:2592])
                    continue
                nc.scalar.activation(
                    os_i[:, g0:g1], ti[:, 0 : g1 - g0], AF.Copy, scale=DESCALE
                )
                nc.vector.tensor_scalar(
                    out=os_j[:, g0:g1], in0=tj[:, 0 : g1 - g0], scalar1=DESCALE,
                    scalar2=None, op0=ALU.mult,
                )
                if last:
                    nc.sync.dma_start(out=res[bsl_i, g0:g1], in_=os_i[:, g0:g1])
                    nc.gpsimd.dma_start(out=res[bsl_j, g0:g1], in_=os_j[:, g0:g1])
            if p < NB // 2 - 1:
                # full-row stores: 5184B HBM segments move faster than the
                # 2048B segments of column-piece stores
                nc.sync.dma_start(out=res[bsl_i, :], in_=os_i[:])
                nc.sync.dma_start(out=res[bsl_j, :], in_=os_j[:])

    nc.finalize()
    return nc


def _pack_host(pose, basis, mask, w1, b1, w2, b2, w3, b3):
    pose_t = pose[:, 1:].reshape(BATCH, 207).T.astype(np.float16)  # [207, B]
    pose_mega = np.zeros((128, 3 * BATCH), np.float16)
    for c, (js, je) in enumerate(CHUNKS):
        K = 9 * (je - js)
        p, hi = divmod(c, 2)
        r0 = 64 if hi else 0
        pose_mega[r0 : r0 + K, p * BATCH : (p + 1) * BATCH] = (
            pose_t[9 * js : 9 * js + K]
        )
        pose_mega[r0 + K, p * BATCH : (p + 1) * BATCH] = 1.0  # ones/bias row

    # bfm rows (j, k) scaled by 2^13, cols (v, c) padded to VPAD.
    bfm = np.zeros((N_JOINT * BPJ, VPAD * 3), np.float32)
    prod = (basis[:, None, :, :] * mask[:, :, None, None] * BSCALE)  # (V, J, K, 3)
    bfm[:, : N_VERT * 3] = prod.transpose(1, 2, 0, 3).reshape(
        N_JOINT * BPJ, N_VERT * 3
    )
    # b3 folded into the main GEMM: bias_vc = b3 . bfm (b-independent).
    bias_vc = (b3.reshape(-1).astype(np.float64) @ bfm.astype(np.float64)).astype(
        np.float32
    )

    w_all = np.zeros((128, W_COLS), np.float16)
    eye9 = np.eye(3, dtype=np.float64).reshape(-1)
    b1f = (
        b1.astype(np.float64) - np.einsum("i,jio->jo", eye9, w1.astype(np.float64))
    ).astype(np.float32)
    for c, ((js, je), o1, o2, o3) in enumerate(zip(CHUNKS, W1_OFF, W2_OFF, W3_OFF)):
        nj = je - js
        r1 = 64 if c % 2 else 0  # odd chunks' W1 blocks live at PE rows 64+
        for t, j in enumerate(range(js, je)):
            w_all[r1 + t * 9 : r1 + (t + 1) * 9, o1 + t * 18 : o1 + (t + 1) * 18] = w1[j]
            w_all[t * 18 : (t + 1) * 18, o2 + t * 32 : o2 + (t + 1) * 32] = w2[j]
            w_all[t * 32 : (t + 1) * 32, o3 + t * 8 : o3 + (t + 1) * 8] = w3[j]
        # bias rows (matched to the activations' ones rows)
        w_all[r1 + 9 * nj, o1 : o1 + 18 * nj] = b1f[js:je].reshape(-1)
        w_all[18 * nj, o2 : o2 + 32 * nj] = b2[js:je].reshape(-1)

    mega = np.concatenate([w_all, pose_mega], axis=1)

    bfm16 = bfm.astype(np.float16)
    bfm_b = np.zeros((128, VPAD * 3), np.float16)
    bfm_b[0:56] = bfm16[128:184]
    bfm_b[56] = bias_vc.astype(np.float16)
    bfm_b[64:120] = bfm16[128:184]
    bfm_b[120] = bias_vc.astype(np.float16)
    return mega, bfm16[0:128], bfm_b


def _in_maps(pose, basis, mask, w1, b1, w2, b2, w3, b3):
    mega, bfm_a, bfm_b = _pack_host(
        np.asarray(pose, np.float32),
        np.asarray(basis, np.float32),
        np.asarray(mask, np.float32),
        np.asarray(w1, np.float32),
        np.asarray(b1, np.float32),
        np.asarray(w2, np.float32),
        np.asarray(b2, np.float32),
        np.asarray(w3, np.float32),
        np.asarray(b3, np.float32),
    )
    ones = np.ones((1, 6 * BATCH), np.float16)
    maps = []
    for i in range(8):
        c0 = i * VC3
        maps.append(
            {
                "mega": mega,
                "bfm_a": np.ascontiguousarray(bfm_a[:, c0 : c0 + VC3]),
                "bfm_b": np.ascontiguousarray(bfm_b[:, c0 : c0 + VC3]),
                "ones": ones,
            }
        )
    return maps


def kernel(pose, basis, mask, w1, b1, w2, b2, w3, b3):
    from concourse.bass_utils import run_bass_kernel_spmd

    if "nc" not in _CACHED:
        _CACHED["nc"] = _build_nc()
    nc = _CACHED["nc"]

    maps = _in_maps(pose, basis, mask, w1, b1, w2, b2, w3, b3)
    r = run_bass_kernel_spmd(nc, maps, core_ids=list(range(8)))
    out = np.concatenate(
        [m["res"].astype(np.float32).reshape(BATCH, VC, 3) for m in r.results],
        axis=1,
    )
    return np.ascontiguousarray(out[:, :N_VERT, :])


# revision 23
# speedup vs baseline: 1.0649x; 1.0649x over previous
"""BlendShapes model kernel for 8 Trainium2 NeuronCores (warm-PE design).

Computation (reference):
    pose_repr = pose[:, 1:].reshape(B, 23, 9) - eye      # (B, J, 9)
    per-joint MLP 9 -> 18 -> 32 -> 8 (ReLU between)      # coff (B, J, 8)
    basis_full = basis[:, None] * mask[:, :, None, None]  # (V, J, 8, 3)
    res = einsum('bjk,vjkc->bvc', coff, basis_full)       # (B, V, 3)

Mapping (per core; vertices sharded 8 ways, V=6890 padded to 8*864):
  - Host precomputes bfm = basis*mask*2^13 (f16, rows (j,k), cols (v,c)).
    ALL biases are folded into matmuls so every PSUM evacuation is a pure
    ReLU / scaled-copy (runs on either ACT or DVE, no bias operand):
      * eye-subtraction -> L1 bias (b1' = b1 - e @ W1)
      * b1', b2 -> ones-row trick (activations carry a constant-1 row,
        weights carry the bias as an extra contraction row)
      * b3 -> folded into the main GEMM: bias_vc = b3 . bfm is a
        b-independent column vector, added via a ones-row in the K=57
        B-pass (coffT_b row 56 = 1, bfm_b row 56 = bias_vc).
  - The PE's HAM clock gate throttles matmuls to 1.2 GHz until ~6us of
    sustained activity, then 2.4 GHz. Warm-up matmuls run during the input
    DMAs; "pre-matmuls" into each upcoming PSUM tile keep the PE dense
    through the MLP's epilogue-paced stretches.
  - Input DMAs: one mega DMA (w + pose images) on the sync queue; bfm on
    the gpsimd queue fenced behind the mega DMA so its 1.3MB doesn't starve
    the MLP critical path (DMA engines arbitrate between queues in bursts).
  - MLP joint chunks of 4 (6 chunks):
      L1 (K=37, M=72):  chunk pairs row-tiled at PE rows 0 / 64 -> 2x
      L2 (K=73, M=128): plain matmuls
      L3 (K=128, M=32): col-tiled 4-way straight into coffT layout
  - Main GEMM out[b, (v,c)] = coffT.T @ bfm, K=184 split 128+56(+bias row),
    b-tiles in pairs: A-passes, then both K=57 B-passes CONCURRENTLY in PE
    row groups 0 / 64; per-chunk evacuation on ACT (i) and DVE (j).
  - Output stored f16 (descale 2^-13 in the evacuation); host converts.
"""

import numpy as np

N_VERT, N_JOINT, BPJ, BATCH = 6890, 23, 8, 1024
VPAD = 6912  # 8 * 864
VC = VPAD // 8  # 864 vertices per core
VC3 = VC * 3  # 2592
NB = BATCH // 128  # 8 b-tiles
NT_BOUNDS = [0, 512, 1024, 1536, 2048, 2560, 2592]
PAIR_BOUNDS = [0, 1024, 2048, 2592]

CHUNKS = [(0, 4), (4, 8), (8, 12), (12, 16), (16, 20), (20, 23)]


def _offsets(mpj):
    offs, col = [], 0
    for js, je in CHUNKS:
        offs.append(col)
        col += (je - js) * mpj
    return offs, col


W1_OFF, W1_TOT = _offsets(18)  # 414
W2_OFF, W2_TOT = _offsets(32)  # 736
W3_OFF, W3_TOT = _offsets(8)   # 184
W2_OFF = [W1_TOT + o for o in W2_OFF]
W3_OFF = [W1_TOT + W2_TOT + o for o in W3_OFF]
W_COLS = W1_TOT + W2_TOT + W3_TOT  # 1334

BSCALE = 8192.0  # 2**13, exact in f16/f32
DESCALE = 1.0 / 8192.0
N_WARMUP = 11  # warm-up matmuls (N=512) before the MLP

_CACHED = {}


def _build_nc():
    import concourse.tile as tile
    from concourse import bacc, mybir
    from contextlib import ExitStack

    dt = mybir.dt
    f32, f16 = dt.float32, dt.float16
    AF = mybir.ActivationFunctionType
    ALU = mybir.AluOpType

    nc = bacc.Bacc(None, target_bir_lowering=False)

    # mega input: [128, W_COLS + 3*1024] f16 = w_all columns followed by the
    # three pose-pair tile images (chunk 2p at rows 0.., 2p+1 at rows 64..,
    # each with its constant-1 bias row baked in).
    MEGA_COLS = W_COLS + 3 * BATCH
    mega_d = nc.dram_tensor("mega", [128, MEGA_COLS], f16, kind="ExternalInput")
    bfm_a_d = nc.dram_tensor("bfm_a", [128, VC3], f16, kind="ExternalInput")
    # bfm_b rows: 0-55 data, 56 bias_vc, 64-119 data dup, 120 bias_vc dup
    # (the B-pass runs two b-tiles concurrently in PE row groups 0 and 64).
    bfm_b_d = nc.dram_tensor("bfm_b", [128, VC3], f16, kind="ExternalInput")
    ones_d = nc.dram_tensor("ones", [1, 6 * BATCH], f16, kind="ExternalInput")
    res = nc.dram_tensor("res", [BATCH, VC3], f16, kind="ExternalOutput")

    with ExitStack() as ctx:
        tc = ctx.enter_context(tile.TileContext(nc))
        const = ctx.enter_context(tc.tile_pool(name="const", bufs=1))
        work = ctx.enter_context(tc.tile_pool(name="work", bufs=1))
        outp = ctx.enter_context(tc.tile_pool(name="outp", bufs=4))
        psum = ctx.enter_context(tc.tile_pool(name="psum", bufs=4, space="PSUM"))

        # warm-up source: memset on DVE (its queue opens early); a tiny
        # ACTIVATE right away pulls the 1.3us ACT table load off the
        # critical path.
        warm = work.tile([128, 512], f16, tag="warm")
        nc.vector.memset(warm[:], 0.0)
        actwarm = work.tile([128, 16], f16, tag="actwarm")
        nc.scalar.activation(actwarm[0:1, :], warm[0:1, 0:16], AF.Relu, bias=0.0)

        # ---- input DMAs.
        mega = const.tile([128, MEGA_COLS], f16, tag="mega")
        nc.sync.dma_start(out=mega[:], in_=mega_d[:, :])
        w_sb = mega[:, 0:W_COLS]
        pose_p = [
            mega[:, W_COLS + p * BATCH : W_COLS + (p + 1) * BATCH] for p in range(3)
        ]

        # h1_all: columns 1024c hold chunk c's activations; row 72 (row 54
        # for chunk 5) carries the constant-1 for the L2 ones-row bias.
        h1_all = work.tile([128, 6 * BATCH], f16, tag="h1_all")
        coffT_a = work.tile([128, BATCH], f16, tag="coffT_a")
        coffT_b = work.tile([128, BATCH], f16, tag="coffT_b")
        nc.gpsimd.dma_start(out=h1_all[72:73, :], in_=ones_d[0:1, :])
        nc.gpsimd.dma_start(
            out=h1_all[54:55, 5 * BATCH : 6 * BATCH], in_=ones_d[0:1, 0:BATCH]
        )
        nc.gpsimd.dma_start(out=coffT_b[56:57, :], in_=ones_d[0:1, 0:BATCH])
        nc.gpsimd.dma_start(out=coffT_b[120:121, :], in_=ones_d[0:1, 0:BATCH])

        # fence: a byte in each bfm tile that depends on the mega data, so
        # the bfm DMAs (WAW) can't start until the mega DMA completed.
        bfm_a = work.tile([128, VC3], f16, tag="bfm_a")
        bfm_b = work.tile([128, VC3], f16, tag="bfm_b")
        nc.gpsimd.tensor_scalar(
            out=bfm_a[64:65, 0:1], in0=pose_p[2][64:65, 1023:1024], scalar1=1.0,
            scalar2=None, op0=ALU.mult,
        )
        nc.gpsimd.tensor_scalar(
            out=bfm_b[32:33, 0:1], in0=pose_p[2][64:65, 1022:1023], scalar1=1.0,
            scalar2=None, op0=ALU.mult,
        )
        nc.gpsimd.dma_start(out=bfm_a[:], in_=bfm_a_d[:, :])
        nc.gpsimd.dma_start(out=bfm_b[:], in_=bfm_b_d[:, :])

        # ---- PSUM allocation with a reserved filler buffer: the pool
        # rotates 4 bufs; whenever the rotation would hand buf0 to a real
        # tile, a dummy "fill" tile takes that slot instead. Filler matmuls
        # target the fill tile, so they NEVER wait on real-tile evacuations
        # and can keep the PE's HAM activity window busy during the MLP's
        # epilogue-paced stretches. The main loop allocates its tiles
        # through ps_tile too but emits no fillers (its stream is dense).
        alloc_ctr = [0]
        fill = [None]

        def ps_tile(name, reserve=True):
            if reserve and alloc_ctr[0] % 4 == 0:
                fill[0] = psum.tile(
                    [128, 1024], f32, tag="ps", name=f"fill_{alloc_ctr[0]}"
                )
                alloc_ctr[0] += 1
            t = psum.tile([128, 1024], f32, tag="ps", name=name)
            alloc_ctr[0] += 1
            return t

        def filler(n=1):
            for _ in range(n):
                nc.tensor.matmul(
                    fill[0][:, 0:512], lhsT=warm[:, 0:128], rhs=warm[:],
                    start=True, stop=True, skip_group_check=True,
                )

        # ---- PE warm-up.
        fill[0] = psum.tile([128, 1024], f32, tag="ps", name="warm_ps")
        alloc_ctr[0] = 1
        filler(N_WARMUP)

        ep_ctr = [0]

        def epilogue(dst, src, relu=False, scale=None):
            # pure ReLU / copy -- either engine; alternate for balance.
            e = ep_ctr[0] % 2
            ep_ctr[0] += 1
            if e == 0:
                if relu:
                    nc.scalar.activation(dst, src, AF.Relu, bias=0.0)
                else:
                    nc.scalar.activation(
                        dst, src, AF.Copy, scale=1.0 if scale is None else scale
                    )
            elif relu:
                nc.vector.tensor_scalar(
                    out=dst, in0=src, scalar1=0.0, scalar2=None, op0=ALU.max
                )
            else:
                nc.vector.tensor_scalar(
                    out=dst, in0=src, scalar1=1.0 if scale is None else scale,
                    scalar2=None, op0=ALU.mult,
                )

        h2 = {}
        HALVES = (slice(0, 512), slice(512, 1024))

        def KM1(c):
            nj = CHUNKS[c][1] - CHUNKS[c][0]
            return 9 * nj + 1, 18 * nj  # +1: ones/bias row

        # L1: row-tiled chunk pairs (rows 0 / 64), both halves of B, one
        # [*,1024] PSUM tile per chunk -> one pure-ReLU epilogue per chunk.
        for p in range(3):
            c0, c1 = 2 * p, 2 * p + 1
            K0, M0 = KM1(c0)
            K1, M1 = KM1(c1)
            ps0 = ps_tile(f"ps1_{c0}")
            ps1 = ps_tile(f"ps1_{c1}")
            for h, hs in enumerate(HALVES):
                nc.tensor.matmul(
                    ps0[0:M0, hs], lhsT=w_sb[0:K0, W1_OFF[c0] : W1_OFF[c0] + M0],
                    rhs=pose_p[p][0:K0, hs], start=True, stop=True,
                    tile_position=(0, 0),
                )
                nc.tensor.matmul(
                    ps1[0:M1, hs], lhsT=w_sb[64 : 64 + K1, W1_OFF[c1] : W1_OFF[c1] + M1],
                    rhs=pose_p[p][64 : 64 + K1, hs], start=True, stop=True,
                    tile_position=(64, 0),
                )
            epilogue(h1_all[0:M0, c0 * BATCH : (c0 + 1) * BATCH], ps0[0:M0, :], relu=True)
            epilogue(h1_all[0:M1, c1 * BATCH : (c1 + 1) * BATCH], ps1[0:M1, :], relu=True)
            filler(2)

        # L2: plain per-chunk matmuls (K includes the ones/bias row).
        for c, (js, je) in enumerate(CHUNKS):
            nj = je - js
            K, M = 18 * nj + 1, 32 * nj
            h2[c] = work.tile([M, BATCH], f16, tag=f"h2_{c}", name=f"h2_{c}")
            ps = ps_tile(f"ps2_{c}")
            for h, hs in enumerate(HALVES):
                nc.tensor.matmul(
                    ps[0:M, hs], lhsT=w_sb[0:K, W2_OFF[c] : W2_OFF[c] + M],
                    rhs=h1_all[0:K, c * BATCH + hs.start : c * BATCH + hs.stop],
                    start=True, stop=True,
                )
            epilogue(h2[c][:, :], ps[0:M, :], relu=True)
            filler(2)

        # L3: col-tiled into coffT layout; pure-copy epilogues (b3 is folded
        # into the main B-pass via bias_vc). Group B lands twice (cols 0/32
        # and 64/96) for the concurrent B-passes.
        psA = ps_tile("ps3a")
        for h, hs in enumerate(HALVES):
            for c in range(4):
                nc.tensor.matmul(
                    psA[32 * c : 32 * c + 32, hs],
                    lhsT=w_sb[0:128, W3_OFF[c] : W3_OFF[c] + 32],
                    rhs=h2[c][:, hs], start=True, stop=True,
                    tile_position=(0, 32 * c),
                )
        epilogue(coffT_a[:, :], psA[:, :])
        filler(3)
        psB = ps_tile("ps3b")
        for h, hs in enumerate(HALVES):
            for r in (0, 64):
                nc.tensor.matmul(
                    psB[r : r + 32, hs], lhsT=w_sb[0:128, W3_OFF[4] : W3_OFF[4] + 32],
                    rhs=h2[4][:, hs], start=True, stop=True, tile_position=(0, r),
                )
                nc.tensor.matmul(
                    psB[r + 32 : r + 56, hs], lhsT=w_sb[0:96, W3_OFF[5] : W3_OFF[5] + 24],
                    rhs=h2[5][:, hs], start=True, stop=True, tile_position=(0, r + 32),
                )
        epilogue(coffT_b[0:56, :], psB[0:56, :])
        epilogue(coffT_b[64:120, :], psB[64:120, :])
        filler(4)

        # ---- main GEMM, b-tiles in pairs: per 1024-wide N-chunk, A-passes
        # (K=128) for both b-tiles, then the two K=57 B-passes (bias row
        # included) CONCURRENTLY in PE row groups 0 / 64; evacuation with
        # the 2^-13 descale on ACT (tile i) and DVE (tile j) in parallel.
        for p in range(NB // 2):
            bti, btj = 2 * p, 2 * p + 1
            bsl_i = slice(bti * 128, bti * 128 + 128)
            bsl_j = slice(btj * 128, btj * 128 + 128)
            os_i = outp.tile([128, VC3], f16, tag="ostrip", name=f"ostrip_{bti}")
            os_j = outp.tile([128, VC3], f16, tag="ostrip", name=f"ostrip_{btj}")
            for g in range(3):
                g0, g1 = PAIR_BOUNDS[g], PAIR_BOUNDS[g + 1]
                ti = ps_tile(f"psm_{p}_{g}_i", reserve=False)
                tj = ps_tile(f"psm_{p}_{g}_j", reserve=False)
                subs = [
                    (slice(n0 - g0, n1 - g0), slice(n0, n1))
                    for n0, n1 in zip(NT_BOUNDS, NT_BOUNDS[1:])
                    if g0 <= n0 < g1
                ]
                for ps, bsl in ((ti, bsl_i), (tj, bsl_j)):
                    for ssl, nsl in subs:
                        nc.tensor.matmul(
                            ps[:, ssl], lhsT=coffT_a[:, bsl], rhs=bfm_a[:, nsl],
                            start=True, stop=False,
                        )
                for ssl, nsl in subs:
                    nc.tensor.matmul(
                        ti[:, ssl], lhsT=coffT_b[0:57, bsl_i],
                        rhs=bfm_b[0:57, nsl], start=False, stop=True,
                        tile_position=(0, 0),
                    )
                    nc.tensor.matmul(
                        tj[:, ssl], lhsT=coffT_b[64:121, bsl_j],
                        rhs=bfm_b[64:121, nsl], start=False, stop=True,
                        tile_position=(64, 0),
                    )
                last = p == NB // 2 - 1
                if last and g == 2:
                    # final chunk: evacuate + store 512 then 32 cols so the
                    # very last transfer is tiny (short tail)
                    nc.scalar.activation(
                        os_i[:, 2048:2560], ti[:, 0:512], AF.Copy, scale=DESCALE
                    )
                    nc.vector.tensor_scalar(
                        out=os_j[:, 2048:2560], in0=tj[:, 0:512], scalar1=DESCALE,
                        scalar2=None, op0=ALU.mult,
                    )
                    nc.sync.dma_start(out=res[bsl_i, 2048:2560], in_=os_i[:, 2048:2560])
                    nc.gpsimd.dma_start(out=res[bsl_j, 2048:2560], in_=os_j[:, 2048:2560])
                    nc.scalar.activation(
                        os_i[:, 2560:2592], ti[:, 512:544], AF.Copy, scale=DESCALE
                    )
                    nc.vector.tensor_scalar(
                        out=os_j[:, 2560:2592], in0=tj[:, 512:544], scalar1=DESCALE,
                        scalar2=None, op0=ALU.mult,
                    )
                    nc.sync.dma_start(out=res[bsl_i, 2560:2592], in_=os_i[:, 2560:2592])
                    nc.gpsimd.dma_start(out=res[bsl_j, 2560:2592], in_=os_j[:, 2560:2592])
                    continue
                nc.scalar.activation(
                    os_i[:, g0:g1], ti[:, 0 : g1 - g0], AF.Copy, scale=DESCALE
                )
                nc.vector.tensor_scalar(
                    out=os_j[:, g0:g1], in0=tj[:, 0 : g1 - g0], scalar1=DESCALE,
                    scalar2=None, op0=ALU.mult,
                )
                if last:
                    nc.sync.dma_start(out=res[bsl_i, g0:g1], in_=os_i[:, g0:g1])
                    nc.gpsimd.dma_start(out=res[bsl_j, g0:g1], in_=os_j[:, g0:g1])
            if p < NB // 2 - 1:
                # full-row stores: 5184B HBM segments move faster than the
                # 2048B segments of column-piece stores
                nc.sync.dma_start(out=res[bsl_i, :], in_=os_i[:])
                nc.sync.dma_start(out=res[bsl_j, :], in_=os_j[:])

    nc.finalize()
    return nc


def _pack_host(pose, basis, mask, w1, b1, w2, b2, w3, b3):
    pose_t = pose[:, 1:].reshape(BATCH, 207).T.astype(np.float16)  # [207, B]
    pose_mega = np.zeros((128, 3 * BATCH), np.float16)
    for c, (js, je) in enumerate(CHUNKS):
        K = 9 * (je - js)
        p, hi = divmod(c, 2)
        r0 = 64 if hi else 0
        pose_mega[r0 : r0 + K, p * BATCH : (p + 1) * BATCH] = (
            pose_t[9 * js : 9 * js + K]
        )
        pose_mega[r0 + K, p * BATCH : (p + 1) * BATCH] = 1.0  # ones/bias row

    # bfm rows (j, k) scaled by 2^13, cols (v, c) padded to VPAD.
    bfm = np.zeros((N_JOINT * BPJ, VPAD * 3), np.float32)
    prod = (basis[:, None, :, :] * mask[:, :, None, None] * BSCALE)  # (V, J, K, 3)
    bfm[:, : N_VERT * 3] = prod.transpose(1, 2, 0, 3).reshape(
        N_JOINT * BPJ, N_VERT * 3
    )
    # b3 folded into the main GEMM: bias_vc = b3 . bfm (b-independent).
    bias_vc = (b3.reshape(-1).astype(np.float64) @ bfm.astype(np.float64)).astype(
        np.float32
    )

    w_all = np.zeros((128, W_COLS), np.float16)
    eye9 = np.eye(3, dtype=np.float64).reshape(-1)
    b1f = (
        b1.astype(np.float64) - np.einsum("i,jio->jo", eye9, w1.astype(np.float64))
    ).astype(np.float32)
    for c, ((js, je), o1, o2, o3) in enumerate(zip(CHUNKS, W1_OFF, W2_OFF, W3_OFF)):
        nj = je - js
        r1 = 64 if c % 2 else 0  # odd chunks' W1 blocks live at PE rows 64+
        for t, j in enumerate(range(js, je)):
            w_all[r1 + t * 9 : r1 + (t + 1) * 9, o1 + t * 18 : o1 + (t + 1) * 18] = w1[j]
            w_all[t * 18 : (t + 1) * 18, o2 + t * 32 : o2 + (t + 1) * 32] = w2[j]
            w_all[t * 32 : (t + 1) * 32, o3 + t * 8 : o3 + (t + 1) * 8] = w3[j]
        # bias rows (matched to the activations' ones rows)
        w_all[r1 + 9 * nj, o1 : o1 + 18 * nj] = b1f[js:je].reshape(-1)
        w_all[18 * nj, o2 : o2 + 32 * nj] = b2[js:je].reshape(-1)

    mega = np.concatenate([w_all, pose_mega], axis=1)

    bfm16 = bfm.astype(np.float16)
    bfm_b = np.zeros((128, VPAD * 3), np.float16)
    bfm_b[0:56] = bfm16[128:184]
    bfm_b[56] = bias_vc.astype(np.float16)
    bfm_b[64:120] = bfm16[128:184]
    bfm_b[120] = bias_vc.astype(np.float16)
    return mega, bfm16[0:128], bfm_b


def _in_maps(pose, basis, mask, w1, b1, w2, b2, w3, b3):
    mega, bfm_a, bfm_b = _pack_host(
        np.asarray(pose, np.float32),
        np.asarray(basis, np.float32),
        np.asarray(mask, np.float32),
        np.asarray(w1, np.float32),
        np.asarray(b1, np.float32),
        np.asarray(w2, np.float32),
        np.asarray(b2, np.float32),
        np.asarray(w3, np.float32),
        np.asarray(b3, np.float32),
    )
    ones = np.ones((1, 6 * BATCH), np.float16)
    maps = []
    for i in range(8):
        c0 = i * VC3
        maps.append(
            {
                "mega": mega,
                "bfm_a": np.ascontiguousarray(bfm_a[:, c0 : c0 + VC3]),
                "bfm_b": np.ascontiguousarray(bfm_b[:, c0 : c0 + VC3]),
                "ones": ones,
            }
        )
    return maps


def kernel(pose, basis, mask, w1, b1, w2, b2, w3, b3):
    from concourse.bass_utils import run_bass_kernel_spmd

    if "nc" not in _CACHED:
        _CACHED["nc"] = _build_nc()
    nc = _CACHED["nc"]

    maps = _in_maps(pose, basis, mask, w1, b1, w2, b2, w3, b3)
    r = run_bass_kernel_spmd(nc, maps, core_ids=list(range(8)))
    out = np.concatenate(
        [m["res"].astype(np.float32).reshape(BATCH, VC, 3) for m in r.results],
        axis=1,
    )
    return np.ascontiguousarray(out[:, :N_VERT, :])


# revision 24
# speedup vs baseline: 1.1341x; 1.0650x over previous
"""BlendShapes model kernel for 8 Trainium2 NeuronCores (warm-PE design).

Computation (reference):
    pose_repr = pose[:, 1:].reshape(B, 23, 9) - eye      # (B, J, 9)
    per-joint MLP 9 -> 18 -> 32 -> 8 (ReLU between)      # coff (B, J, 8)
    basis_full = basis[:, None] * mask[:, :, None, None]  # (V, J, 8, 3)
    res = einsum('bjk,vjkc->bvc', coff, basis_full)       # (B, V, 3)

Mapping (per core; vertices sharded 8 ways, V=6890 padded to 8*864):
  - Host precomputes bfm = basis*mask*2^13 (f16, rows (j,k), cols (v,c)).
    ALL biases are folded into matmuls so every PSUM evacuation is a pure
    ReLU / scaled-copy (runs on either ACT or DVE, no bias operand):
      * eye-subtraction -> L1 bias (b1' = b1 - e @ W1)
      * b1', b2 -> ones-row trick (activations carry a constant-1 row,
        weights carry the bias as an extra contraction row)
      * b3 -> folded into the main GEMM: bias_vc = b3 . bfm is a
        b-independent column vector, added via a ones-row in the K=57
        B-pass (coffT_b row 56 = 1, bfm_b row 56 = bias_vc).
  - The PE's HAM clock gate throttles matmuls to 1.2 GHz until ~6us of
    sustained activity, then 2.4 GHz. Warm-up matmuls run during the input
    DMAs; "pre-matmuls" into each upcoming PSUM tile keep the PE dense
    through the MLP's epilogue-paced stretches.
  - Input DMAs: one mega DMA (w + pose images) on the sync queue; bfm on
    the gpsimd queue fenced behind the mega DMA so its 1.3MB doesn't starve
    the MLP critical path (DMA engines arbitrate between queues in bursts).
  - MLP joint chunks of 4 (6 chunks):
      L1 (K=37, M=72):  chunk pairs row-tiled at PE rows 0 / 64 -> 2x
      L2 (K=73, M=128): plain matmuls
      L3 (K=128, M=32): col-tiled 4-way straight into coffT layout
  - Main GEMM out[b, (v,c)] = coffT.T @ bfm, K=184 split 128+56(+bias row),
    b-tiles in pairs: A-passes, then both K=57 B-passes CONCURRENTLY in PE
    row groups 0 / 64; per-chunk evacuation on ACT (i) and DVE (j).
  - Output stored f16 (descale 2^-13 in the evacuation); host converts.
"""

import numpy as np

N_VERT, N_JOINT, BPJ, BATCH = 6890, 23, 8, 1024
VPAD = 6912  # 8 * 864
VC = VPAD // 8  # 864 vertices per core
VC3 = VC * 3  # 2592
NB = BATCH // 128  # 8 b-tiles
NT_BOUNDS = [0, 512, 1024, 1536, 2048, 2560, 2592]
PAIR_BOUNDS = [0, 1024, 2048, 2592]

CHUNKS = [(0, 4), (4, 8), (8, 12), (12, 16), (16, 20), (20, 23)]


def _offsets(mpj):
    offs, col = [], 0
    for js, je in CHUNKS:
        offs.append(col)
        col += (je - js) * mpj
    return offs, col


W1_OFF, W1_TOT = _offsets(18)  # 414
W2_OFF, W2_TOT = _offsets(32)  # 736
W3_OFF, W3_TOT = _offsets(8)   # 184
W2_OFF = [W1_TOT + o for o in W2_OFF]
W3_OFF = [W1_TOT + W2_TOT + o for o in W3_OFF]
W_COLS = W1_TOT + W2_TOT + W3_TOT  # 1334

BSCALE = 8192.0  # 2**13, exact in f16/f32
DESCALE = 1.0 / 8192.0
N_WARMUP = 11  # warm-up matmuls (N=512) before the MLP

_CACHED = {}


def _build_nc():
    import concourse.tile as tile
    from concourse import bacc, mybir
    from contextlib import ExitStack

    dt = mybir.dt
    f32, f16 = dt.float32, dt.float16
    AF = mybir.ActivationFunctionType
    ALU = mybir.AluOpType

    nc = bacc.Bacc(None, target_bir_lowering=False)

    # mega input: [128, W_COLS + 3*1024] f16 = w_all columns followed by the
    # three pose-pair tile images (chunk 2p at rows 0.., 2p+1 at rows 64..,
    # each with its constant-1 bias row baked in).
    MEGA_COLS = W_COLS + 3 * BATCH
    mega_d = nc.dram_tensor("mega", [128, MEGA_COLS], f16, kind="ExternalInput")
    bfm_a_d = nc.dram_tensor("bfm_a", [128, VC3], f16, kind="ExternalInput")
    # bfm_b rows: 0-55 data, 56 bias_vc, 64-119 data dup, 120 bias_vc dup
    # (the B-pass runs two b-tiles concurrently in PE row groups 0 and 64).
    bfm_b_d = nc.dram_tensor("bfm_b", [128, VC3], f16, kind="ExternalInput")
    ones_d = nc.dram_tensor("ones", [1, 6 * BATCH], f16, kind="ExternalInput")
    res = nc.dram_tensor("res", [BATCH, VC3], f16, kind="ExternalOutput")

    with ExitStack() as ctx:
        tc = ctx.enter_context(tile.TileContext(nc))
        const = ctx.enter_context(tc.tile_pool(name="const", bufs=1))
        work = ctx.enter_context(tc.tile_pool(name="work", bufs=1))
        outp = ctx.enter_context(tc.tile_pool(name="outp", bufs=4))
        psum = ctx.enter_context(tc.tile_pool(name="psum", bufs=4, space="PSUM"))

        # warm-up source: memset on DVE (its queue opens early); a tiny
        # ACTIVATE right away pulls the 1.3us ACT table load off the
        # critical path.
        warm = work.tile([128, 512], f16, tag="warm")
        nc.vector.memset(warm[:], 0.0)
        actwarm = work.tile([128, 16], f16, tag="actwarm")
        nc.scalar.activation(actwarm[0:1, :], warm[0:1, 0:16], AF.Relu, bias=0.0)

        # ---- input DMAs.
        mega = const.tile([128, MEGA_COLS], f16, tag="mega")
        nc.sync.dma_start(out=mega[:], in_=mega_d[:, :])
        w_sb = mega[:, 0:W_COLS]
        pose_p = [
            mega[:, W_COLS + p * BATCH : W_COLS + (p + 1) * BATCH] for p in range(3)
        ]

        # h1_all: columns 1024c hold chunk c's activations; row 72 (row 54
        # for chunk 5) carries the constant-1 for the L2 ones-row bias.
        h1_all = work.tile([128, 6 * BATCH], f16, tag="h1_all")
        coffT_a = work.tile([128, BATCH], f16, tag="coffT_a")
        coffT_b = work.tile([128, BATCH], f16, tag="coffT_b")
        nc.gpsimd.dma_start(out=h1_all[72:73, :], in_=ones_d[0:1, :])
        nc.gpsimd.dma_start(
            out=h1_all[54:55, 5 * BATCH : 6 * BATCH], in_=ones_d[0:1, 0:BATCH]
        )
        nc.gpsimd.dma_start(out=coffT_b[56:57, :], in_=ones_d[0:1, 0:BATCH])
        nc.gpsimd.dma_start(out=coffT_b[120:121, :], in_=ones_d[0:1, 0:BATCH])

        # fence: a byte in each bfm tile that depends on the mega data, so
        # the bfm DMAs (WAW) can't start until the mega DMA completed.
        bfm_a = work.tile([128, VC3], f16, tag="bfm_a")
        bfm_b = work.tile([128, VC3], f16, tag="bfm_b")
        nc.gpsimd.tensor_scalar(
            out=bfm_a[64:65, 0:1], in0=pose_p[2][64:65, 1023:1024], scalar1=1.0,
            scalar2=None, op0=ALU.mult,
        )
        nc.gpsimd.tensor_scalar(
            out=bfm_b[32:33, 0:1], in0=pose_p[2][64:65, 1022:1023], scalar1=1.0,
            scalar2=None, op0=ALU.mult,
        )
        nc.gpsimd.dma_start(out=bfm_a[:], in_=bfm_a_d[:, :])
        nc.gpsimd.dma_start(out=bfm_b[:], in_=bfm_b_d[:, :])

        # ---- PSUM allocation with a reserved filler buffer: the pool
        # rotates 4 bufs; whenever the rotation would hand buf0 to a real
        # tile, a dummy "fill" tile takes that slot instead. Filler matmuls
        # target the fill tile, so they NEVER wait on real-tile evacuations
        # and can keep the PE's HAM activity window busy during the MLP's
        # epilogue-paced stretches. The main loop allocates its tiles
        # through ps_tile too but emits no fillers (its stream is dense).
        alloc_ctr = [0]
        fill = [None]

        def ps_tile(name, reserve=True):
            if reserve and alloc_ctr[0] % 4 == 0:
                fill[0] = psum.tile(
                    [128, 1024], f32, tag="ps", name=f"fill_{alloc_ctr[0]}"
                )
                alloc_ctr[0] += 1
            t = psum.tile([128, 1024], f32, tag="ps", name=name)
            alloc_ctr[0] += 1
            return t

        def filler(n=1):
            for _ in range(n):
                nc.tensor.matmul(
                    fill[0][:, 0:512], lhsT=warm[:, 0:128], rhs=warm[:],
                    start=True, stop=True, skip_group_check=True,
                )

        # ---- PE warm-up.
        fill[0] = psum.tile([128, 1024], f32, tag="ps", name="warm_ps")
        alloc_ctr[0] = 1
        filler(N_WARMUP)

        ep_ctr = [0]

        def epilogue(dst, src, relu=False, scale=None):
            # pure ReLU / copy -- either engine; alternate for balance.
            e = ep_ctr[0] % 2
            ep_ctr[0] += 1
            if e == 0:
                if relu:
                    nc.scalar.activation(dst, src, AF.Relu, bias=0.0)
                else:
                    nc.scalar.activation(
                        dst, src, AF.Copy, scale=1.0 if scale is None else scale
                    )
            elif relu:
                nc.vector.tensor_scalar(
                    out=dst, in0=src, scalar1=0.0, scalar2=None, op0=ALU.max
                )
            else:
                nc.vector.tensor_scalar(
                    out=dst, in0=src, scalar1=1.0 if scale is None else scale,
                    scalar2=None, op0=ALU.mult,
                )

        h2 = {}
        HALVES = (slice(0, 512), slice(512, 1024))

        def KM1(c):
            nj = CHUNKS[c][1] - CHUNKS[c][0]
            return 9 * nj + 1, 18 * nj  # +1: ones/bias row

        # L1: row-tiled chunk pairs (rows 0 / 64), both halves of B, one
        # [*,1024] PSUM tile per chunk -> one pure-ReLU epilogue per chunk.
        for p in range(3):
            c0, c1 = 2 * p, 2 * p + 1
            K0, M0 = KM1(c0)
            K1, M1 = KM1(c1)
            ps0 = ps_tile(f"ps1_{c0}")
            ps1 = ps_tile(f"ps1_{c1}")
            for h, hs in enumerate(HALVES):
                nc.tensor.matmul(
                    ps0[0:M0, hs], lhsT=w_sb[0:K0, W1_OFF[c0] : W1_OFF[c0] + M0],
                    rhs=pose_p[p][0:K0, hs], start=True, stop=True,
                    tile_position=(0, 0),
                )
                nc.tensor.matmul(
                    ps1[0:M1, hs], lhsT=w_sb[64 : 64 + K1, W1_OFF[c1] : W1_OFF[c1] + M1],
                    rhs=pose_p[p][64 : 64 + K1, hs], start=True, stop=True,
                    tile_position=(64, 0),
                )
            epilogue(h1_all[0:M0, c0 * BATCH : (c0 + 1) * BATCH], ps0[0:M0, :], relu=True)
            epilogue(h1_all[0:M1, c1 * BATCH : (c1 + 1) * BATCH], ps1[0:M1, :], relu=True)
            filler(2)

        # L2: plain per-chunk matmuls (K includes the ones/bias row).
        for c, (js, je) in enumerate(CHUNKS):
            nj = je - js
            K, M = 18 * nj + 1, 32 * nj
            h2[c] = work.tile([M, BATCH], f16, tag=f"h2_{c}", name=f"h2_{c}")
            ps = ps_tile(f"ps2_{c}")
            for h, hs in enumerate(HALVES):
                nc.tensor.matmul(
                    ps[0:M, hs], lhsT=w_sb[0:K, W2_OFF[c] : W2_OFF[c] + M],
                    rhs=h1_all[0:K, c * BATCH + hs.start : c * BATCH + hs.stop],
                    start=True, stop=True,
                )
            epilogue(h2[c][:, :], ps[0:M, :], relu=True)
            filler(2)

        # L3: col-tiled into coffT layout; pure-copy epilogues (b3 is folded
        # into the main B-pass via bias_vc). Group B lands twice (cols 0/32
        # and 64/96) for the concurrent B-passes. Processed HALF-MAJOR with
        # per-half epilogues: main b-tiles 0-3 need only the half-0
        # coefficients, so the main GEMM starts ~2us earlier and the half-1
        # coff epilogues hide under it.
        psA = ps_tile("ps3a")
        psB = ps_tile("ps3b")
        for h, hs in enumerate(HALVES):
            for c in range(4):
                nc.tensor.matmul(
                    psA[32 * c : 32 * c + 32, hs],
                    lhsT=w_sb[0:128, W3_OFF[c] : W3_OFF[c] + 32],
                    rhs=h2[c][:, hs], start=True, stop=True,
                    tile_position=(0, 32 * c),
                )
            for r in (0, 64):
                nc.tensor.matmul(
                    psB[r : r + 32, hs], lhsT=w_sb[0:128, W3_OFF[4] : W3_OFF[4] + 32],
                    rhs=h2[4][:, hs], start=True, stop=True, tile_position=(0, r),
                )
                nc.tensor.matmul(
                    psB[r + 32 : r + 56, hs], lhsT=w_sb[0:96, W3_OFF[5] : W3_OFF[5] + 24],
                    rhs=h2[5][:, hs], start=True, stop=True, tile_position=(0, r + 32),
                )
            epilogue(coffT_a[:, hs], psA[:, hs])
            epilogue(coffT_b[0:56, hs], psB[0:56, hs])
            epilogue(coffT_b[64:120, hs], psB[64:120, hs])
            filler(2)
        filler(2)
        # ---- main GEMM, b-tiles in pairs: per 1024-wide N-chunk, A-passes
        # (K=128) for both b-tiles, then the two K=57 B-passes (bias row
        # included) CONCURRENTLY in PE row groups 0 / 64; evacuation with
        # the 2^-13 descale on ACT (tile i) and DVE (tile j) in parallel.
        for p in range(NB // 2):
            bti, btj = 2 * p, 2 * p + 1
            bsl_i = slice(bti * 128, bti * 128 + 128)
            bsl_j = slice(btj * 128, btj * 128 + 128)
            os_i = outp.tile([128, VC3], f16, tag="ostrip", name=f"ostrip_{bti}")
            os_j = outp.tile([128, VC3], f16, tag="ostrip", name=f"ostrip_{btj}")
            for g in range(3):
                g0, g1 = PAIR_BOUNDS[g], PAIR_BOUNDS[g + 1]
                ti = ps_tile(f"psm_{p}_{g}_i", reserve=False)
                tj = ps_tile(f"psm_{p}_{g}_j", reserve=False)
                subs = [
                    (slice(n0 - g0, n1 - g0), slice(n0, n1))
                    for n0, n1 in zip(NT_BOUNDS, NT_BOUNDS[1:])
                    if g0 <= n0 < g1
                ]
                for ps, bsl in ((ti, bsl_i), (tj, bsl_j)):
                    for ssl, nsl in subs:
                        nc.tensor.matmul(
                            ps[:, ssl], lhsT=coffT_a[:, bsl], rhs=bfm_a[:, nsl],
                            start=True, stop=False,
                        )
                for ssl, nsl in subs:
                    nc.tensor.matmul(
                        ti[:, ssl], lhsT=coffT_b[0:57, bsl_i],
                        rhs=bfm_b[0:57, nsl], start=False, stop=True,
                        tile_position=(0, 0),
                    )
                    nc.tensor.matmul(
                        tj[:, ssl], lhsT=coffT_b[64:121, bsl_j],
                        rhs=bfm_b[64:121, nsl], start=False, stop=True,
                        tile_position=(64, 0),
                    )
                last = p == NB // 2 - 1
                if last and g == 2:
                    # final chunk: evacuate + store 512 then 32 cols so the
                    # very last transfer is tiny (short tail)
                    nc.scalar.activation(
                        os_i[:, 2048:2560], ti[:, 0:512], AF.Copy, scale=DESCALE
                    )
                    nc.vector.tensor_scalar(
                        out=os_j[:, 2048:2560], in0=tj[:, 0:512], scalar1=DESCALE,
                        scalar2=None, op0=ALU.mult,
                    )
                    nc.sync.dma_start(out=res[bsl_i, 2048:2560], in_=os_i[:, 2048:2560])
                    nc.gpsimd.dma_start(out=res[bsl_j, 2048:2560], in_=os_j[:, 2048:2560])
                    nc.scalar.activation(
                        os_i[:, 2560:2592], ti[:, 512:544], AF.Copy, scale=DESCALE
                    )
                    nc.vector.tensor_scalar(
                        out=os_j[:, 2560:2592], in0=tj[:, 512:544], scalar1=DESCALE,
                        scalar2=None, op0=ALU.mult,
                    )
                    nc.sync.dma_start(out=res[bsl_i, 2560:2592], in_=os_i[:, 2560:2592])
                    nc.gpsimd.dma_start(out=res[bsl_j, 2560:2592], in_=os_j[:, 2560:2592])
                    continue
                nc.scalar.activation(
                    os_i[:, g0:g1], ti[:, 0 : g1 - g0], AF.Copy, scale=DESCALE
                )
                nc.vector.tensor_scalar(
                    out=os_j[:, g0:g1], in0=tj[:, 0 : g1 - g0], scalar1=DESCALE,
                    scalar2=None, op0=ALU.mult,
                )
                if last:
                    nc.sync.dma_start(out=res[bsl_i, g0:g1], in_=os_i[:, g0:g1])
                    nc.gpsimd.dma_start(out=res[bsl_j, g0:g1], in_=os_j[:, g0:g1])
            if p < NB // 2 - 1:
                # full-row stores: 5184B HBM segments move faster than the
                # 2048B segments of column-piece stores
                nc.sync.dma_start(out=res[bsl_i, :], in_=os_i[:])
                nc.sync.dma_start(out=res[bsl_j, :], in_=os_j[:])

    nc.finalize()
    return nc


def _pack_host(pose, basis, mask, w1, b1, w2, b2, w3, b3):
    pose_t = pose[:, 1:].reshape(BATCH, 207).T.astype(np.float16)  # [207, B]
    pose_mega = np.zeros((128, 3 * BATCH), np.float16)
    for c, (js, je) in enumerate(CHUNKS):
        K = 9 * (je - js)
        p, hi = divmod(c, 2)
        r0 = 64 if hi else 0
        pose_mega[r0 : r0 + K, p * BATCH : (p + 1) * BATCH] = (
            pose_t[9 * js : 9 * js + K]
        )
        pose_mega[r0 + K, p * BATCH : (p + 1) * BATCH] = 1.0  # ones/bias row

    # bfm rows (j, k) scaled by 2^13, cols (v, c) padded to VPAD.
    bfm = np.zeros((N_JOINT * BPJ, VPAD * 3), np.float32)
    prod = (basis[:, None, :, :] * mask[:, :, None, None] * BSCALE)  # (V, J, K, 3)
    bfm[:, : N_VERT * 3] = prod.transpose(1, 2, 0, 3).reshape(
        N_JOINT * BPJ, N_VERT * 3
    )
    # b3 folded into the main GEMM: bias_vc = b3 . bfm (b-independent).
    bias_vc = (b3.reshape(-1).astype(np.float64) @ bfm.astype(np.float64)).astype(
        np.float32
    )

    w_all = np.zeros((128, W_COLS), np.float16)
    eye9 = np.eye(3, dtype=np.float64).reshape(-1)
    b1f = (
        b1.astype(np.float64) - np.einsum("i,jio->jo", eye9, w1.astype(np.float64))
    ).astype(np.float32)
    for c, ((js, je), o1, o2, o3) in enumerate(zip(CHUNKS, W1_OFF, W2_OFF, W3_OFF)):
        nj = je - js
        r1 = 64 if c % 2 else 0  # odd chunks' W1 blocks live at PE rows 64+
        for t, j in enumerate(range(js, je)):
            w_all[r1 + t * 9 : r1 + (t + 1) * 9, o1 + t * 18 : o1 + (t + 1) * 18] = w1[j]
            w_all[t * 18 : (t + 1) * 18, o2 + t * 32 : o2 + (t + 1) * 32] = w2[j]
            w_all[t * 32 : (t + 1) * 32, o3 + t * 8 : o3 + (t + 1) * 8] = w3[j]
        # bias rows (matched to the activations' ones rows)
        w_all[r1 + 9 * nj, o1 : o1 + 18 * nj] = b1f[js:je].reshape(-1)
        w_all[18 * nj, o2 : o2 + 32 * nj] = b2[js:je].reshape(-1)

    mega = np.concatenate([w_all, pose_mega], axis=1)

    bfm16 = bfm.astype(np.float16)
    bfm_b = np.zeros((128, VPAD * 3), np.float16)
    bfm_b[0:56] = bfm16[128:184]
    bfm_b[56] = bias_vc.astype(np.float16)
    bfm_b[64:120] = bfm16[128:184]
    bfm_b[120] = bias_vc.astype(np.float16)
    return mega, bfm16[0:128], bfm_b


def _in_maps(pose, basis, mask, w1, b1, w2, b2, w3, b3):
    mega, bfm_a, bfm_b = _pack_host(
        np.asarray(pose, np.float32),
        np.asarray(basis, np.float32),
        np.asarray(mask, np.float32),
        np.asarray(w1, np.float32),
        np.asarray(b1, np.float32),
        np.asarray(w2, np.float32),
        np.asarray(b2, np.float32),
        np.asarray(w3, np.float32),
        np.asarray(b3, np.float32),
    )
    ones = np.ones((1, 6 * BATCH), np.float16)
    maps = []
    for i in range(8):
        c0 = i * VC3
        maps.append(
            {
                "mega": mega,
                "bfm_a": np.ascontiguousarray(bfm_a[:, c0 : c0 + VC3]),
                "bfm_b": np.ascontiguousarray(bfm_b[:, c0 : c0 + VC3]),
                "ones": ones,
            }
        )
    return maps


def kernel(pose, basis, mask, w1, b1, w2, b2, w3, b3):
    from concourse.bass_utils import run_bass_kernel_spmd

    if "nc" not in _CACHED:
        _CACHED["nc"] = _build_nc()
    nc = _CACHED["nc"]

    maps = _in_maps(pose, basis, mask, w1, b1, w2, b2, w3, b3)
    r = run_bass_kernel_spmd(nc, maps, core_ids=list(range(8)))
    out = np.concatenate(
        [m["res"].astype(np.float32).reshape(BATCH, VC, 3) for m in r.results],
        axis=1,
    )
    return np.ascontiguousarray(out[:, :N_VERT, :])


# revision 26
# speedup vs baseline: 1.1765x; 1.0374x over previous
"""BlendShapes model kernel for 8 Trainium2 NeuronCores (warm-PE design).

Computation (reference):
    pose_repr = pose[:, 1:].reshape(B, 23, 9) - eye      # (B, J, 9)
    per-joint MLP 9 -> 18 -> 32 -> 8 (ReLU between)      # coff (B, J, 8)
    basis_full = basis[:, None] * mask[:, :, None, None]  # (V, J, 8, 3)
    res = einsum('bjk,vjkc->bvc', coff, basis_full)       # (B, V, 3)

Mapping (per core; vertices sharded 8 ways, V=6890 padded to 8*864):
  - Host precomputes bfm = basis*mask*2^13 (f16, rows (j,k), cols (v,c)).
    ALL biases are folded into matmuls so every PSUM evacuation is a pure
    ReLU / scaled-copy (runs on either ACT or DVE, no bias operand):
      * eye-subtraction -> L1 bias (b1' = b1 - e @ W1)
      * b1', b2 -> ones-row trick (activations carry a constant-1 row,
        weights carry the bias as an extra contraction row)
      * b3 -> folded into the main GEMM: bias_vc = b3 . bfm is a
        b-independent column vector, added via a ones-row in the K=57
        B-pass (coffT_b row 56 = 1, bfm_b row 56 = bias_vc).
  - The PE's HAM clock gate throttles matmuls to 1.2 GHz until ~6us of
    sustained activity, then 2.4 GHz. Warm-up matmuls run during the input
    DMAs; "pre-matmuls" into each upcoming PSUM tile keep the PE dense
    through the MLP's epilogue-paced stretches.
  - Input DMAs: one mega DMA (w + pose images) on the sync queue; bfm on
    the gpsimd queue fenced behind the mega DMA so its 1.3MB doesn't starve
    the MLP critical path (DMA engines arbitrate between queues in bursts).
  - MLP joint chunks of 4 (6 chunks):
      L1 (K=37, M=72):  chunk pairs row-tiled at PE rows 0 / 64 -> 2x
      L2 (K=73, M=128): plain matmuls
      L3 (K=128, M=32): col-tiled 4-way straight into coffT layout
  - Main GEMM out[b, (v,c)] = coffT.T @ bfm, K=184 split 128+56(+bias row),
    b-tiles in pairs: A-passes, then both K=57 B-passes CONCURRENTLY in PE
    row groups 0 / 64; per-chunk evacuation on ACT (i) and DVE (j).
  - Output stored f16 (descale 2^-13 in the evacuation); host converts.
"""

import numpy as np

N_VERT, N_JOINT, BPJ, BATCH = 6890, 23, 8, 1024
VPAD = 6912  # 8 * 864
VC = VPAD // 8  # 864 vertices per core
VC3 = VC * 3  # 2592
NB = BATCH // 128  # 8 b-tiles
NT_BOUNDS = [0, 512, 1024, 1536, 2048, 2560, 2592]
PAIR_BOUNDS = [0, 1024, 2048, 2592]

CHUNKS = [(0, 4), (4, 8), (8, 12), (12, 16), (16, 20), (20, 23)]


def _offsets(mpj):
    offs, col = [], 0
    for js, je in CHUNKS:
        offs.append(col)
        col += (je - js) * mpj
    return offs, col


W1_OFF, W1_TOT = _offsets(18)  # 414
W2_OFF, W2_TOT = _offsets(32)  # 736
W3_OFF, W3_TOT = _offsets(8)   # 184
W2_OFF = [W1_TOT + o for o in W2_OFF]
W3_OFF = [W1_TOT + W2_TOT + o for o in W3_OFF]
W_COLS = W1_TOT + W2_TOT + W3_TOT  # 1334

BSCALE = 8192.0  # 2**13, exact in f16/f32
DESCALE = 1.0 / 8192.0
N_WARMUP = 11  # warm-up matmuls (N=512) before the MLP

_CACHED = {}


def _build_nc():
    import concourse.tile as tile
    from concourse import bacc, mybir
    from contextlib import ExitStack

    dt = mybir.dt
    f32, f16 = dt.float32, dt.float16
    AF = mybir.ActivationFunctionType
    ALU = mybir.AluOpType

    nc = bacc.Bacc(None, target_bir_lowering=False)

    # mega input: [128, W_COLS + 3*1024] f16 = w_all columns followed by the
    # three pose-pair tile images (chunk 2p at rows 0.., 2p+1 at rows 64..,
    # each with its constant-1 bias row baked in).
    MEGA_COLS = W_COLS + 3 * BATCH
    mega_d = nc.dram_tensor("mega", [128, MEGA_COLS], f16, kind="ExternalInput")
    bfm_a_d = nc.dram_tensor("bfm_a", [128, VC3], f16, kind="ExternalInput")
    # bfm_b rows: 0-55 data, 56 bias_vc, 64-119 data dup, 120 bias_vc dup
    # (the B-pass runs two b-tiles concurrently in PE row groups 0 and 64).
    bfm_b_d = nc.dram_tensor("bfm_b", [128, VC3], f16, kind="ExternalInput")
    ones_d = nc.dram_tensor("ones", [1, 6 * BATCH], f16, kind="ExternalInput")
    res = nc.dram_tensor("res", [BATCH, VC3], f16, kind="ExternalOutput")

    with ExitStack() as ctx:
        tc = ctx.enter_context(tile.TileContext(nc))
        const = ctx.enter_context(tc.tile_pool(name="const", bufs=1))
        work = ctx.enter_context(tc.tile_pool(name="work", bufs=1))
        outp = ctx.enter_context(tc.tile_pool(name="outp", bufs=4))
        psum = ctx.enter_context(tc.tile_pool(name="psum", bufs=4, space="PSUM"))

        # warm-up source: memset on DVE (its queue opens early); a tiny
        # ACTIVATE right away pulls the 1.3us ACT table load off the
        # critical path.
        warm = work.tile([128, 512], f16, tag="warm")
        nc.vector.memset(warm[:], 0.0)
        actwarm = work.tile([128, 16], f16, tag="actwarm")
        nc.scalar.activation(actwarm[0:1, :], warm[0:1, 0:16], AF.Relu, bias=0.0)

        # ---- input DMAs.
        mega = const.tile([128, MEGA_COLS], f16, tag="mega")
        nc.sync.dma_start(out=mega[:], in_=mega_d[:, :])
        w_sb = mega[:, 0:W_COLS]
        pose_p = [
            mega[:, W_COLS + p * BATCH : W_COLS + (p + 1) * BATCH] for p in range(3)
        ]

        # h1_all: columns 1024c hold chunk c's activations; row 72 (row 54
        # for chunk 5) carries the constant-1 for the L2 ones-row bias.
        h1_all = work.tile([128, 6 * BATCH], f16, tag="h1_all")
        # per-half coefficient tiles: main pairs 0-1 read only the half-0
        # tiles, so they start as soon as the half-0 epilogues land.
        coffT_a = [
            work.tile([128, 512], f16, tag=f"coffT_a{h}", name=f"coffT_a{h}")
            for h in (0, 1)
        ]
        coffT_b = [
            work.tile([128, 512], f16, tag=f"coffT_b{h}", name=f"coffT_b{h}")
            for h in (0, 1)
        ]
        nc.gpsimd.dma_start(out=h1_all[72:73, :], in_=ones_d[0:1, :])
        nc.gpsimd.dma_start(
            out=h1_all[54:55, 5 * BATCH : 6 * BATCH], in_=ones_d[0:1, 0:BATCH]
        )
        for h in (0, 1):
            nc.gpsimd.dma_start(out=coffT_b[h][56:57, :], in_=ones_d[0:1, 0:512])
            nc.gpsimd.dma_start(out=coffT_b[h][120:121, :], in_=ones_d[0:1, 0:512])

        # fence: a byte in each bfm tile that depends on the mega data, so
        # the bfm DMAs (WAW) can't start until the mega DMA completed.
        bfm_a = work.tile([128, VC3], f16, tag="bfm_a")
        bfm_b = work.tile([128, VC3], f16, tag="bfm_b")
        nc.gpsimd.tensor_scalar(
            out=bfm_a[64:65, 0:1], in0=pose_p[2][64:65, 1023:1024], scalar1=1.0,
            scalar2=None, op0=ALU.mult,
        )
        nc.gpsimd.tensor_scalar(
            out=bfm_b[32:33, 0:1], in0=pose_p[2][64:65, 1022:1023], scalar1=1.0,
            scalar2=None, op0=ALU.mult,
        )
        nc.gpsimd.dma_start(out=bfm_a[:], in_=bfm_a_d[:, :])
        nc.gpsimd.dma_start(out=bfm_b[:], in_=bfm_b_d[:, :])

        # ---- PSUM allocation with a reserved filler buffer: the pool
        # rotates 4 bufs; whenever the rotation would hand buf0 to a real
        # tile, a dummy "fill" tile takes that slot instead. Filler matmuls
        # target the fill tile, so they NEVER wait on real-tile evacuations
        # and can keep the PE's HAM activity window busy during the MLP's
        # epilogue-paced stretches. The main loop allocates its tiles
        # through ps_tile too but emits no fillers (its stream is dense).
        alloc_ctr = [0]
        fill = [None]

        def ps_tile(name, reserve=True):
            if reserve and alloc_ctr[0] % 4 == 0:
                fill[0] = psum.tile(
                    [128, 1024], f32, tag="ps", name=f"fill_{alloc_ctr[0]}"
                )
                alloc_ctr[0] += 1
            t = psum.tile([128, 1024], f32, tag="ps", name=name)
            alloc_ctr[0] += 1
            return t

        def filler(n=1):
            for _ in range(n):
                nc.tensor.matmul(
                    fill[0][:, 0:512], lhsT=warm[:, 0:128], rhs=warm[:],
                    start=True, stop=True, skip_group_check=True,
                )

        # ---- PE warm-up.
        fill[0] = psum.tile([128, 1024], f32, tag="ps", name="warm_ps")
        alloc_ctr[0] = 1
        filler(N_WARMUP)

        ep_ctr = [0]

        def epilogue(dst, src, relu=False, scale=None):
            # pure ReLU / copy -- either engine; alternate for balance.
            e = ep_ctr[0] % 2
            ep_ctr[0] += 1
            if e == 0:
                if relu:
                    nc.scalar.activation(dst, src, AF.Relu, bias=0.0)
                else:
                    nc.scalar.activation(
                        dst, src, AF.Copy, scale=1.0 if scale is None else scale
                    )
            elif relu:
                nc.vector.tensor_scalar(
                    out=dst, in0=src, scalar1=0.0, scalar2=None, op0=ALU.max
                )
            else:
                nc.vector.tensor_scalar(
                    out=dst, in0=src, scalar1=1.0 if scale is None else scale,
                    scalar2=None, op0=ALU.mult,
                )

        h2 = {}
        HALVES = (slice(0, 512), slice(512, 1024))

        def KM1(c):
            nj = CHUNKS[c][1] - CHUNKS[c][0]
            return 9 * nj + 1, 18 * nj  # +1: ones/bias row

        # L1: row-tiled chunk pairs (rows 0 / 64), both halves of B, one
        # [*,1024] PSUM tile per chunk -> one pure-ReLU epilogue per chunk.
        for p in range(3):
            c0, c1 = 2 * p, 2 * p + 1
            K0, M0 = KM1(c0)
            K1, M1 = KM1(c1)
            ps0 = ps_tile(f"ps1_{c0}")
            ps1 = ps_tile(f"ps1_{c1}")
            for h, hs in enumerate(HALVES):
                nc.tensor.matmul(
                    ps0[0:M0, hs], lhsT=w_sb[0:K0, W1_OFF[c0] : W1_OFF[c0] + M0],
                    rhs=pose_p[p][0:K0, hs], start=True, stop=True,
                    tile_position=(0, 0),
                )
                nc.tensor.matmul(
                    ps1[0:M1, hs], lhsT=w_sb[64 : 64 + K1, W1_OFF[c1] : W1_OFF[c1] + M1],
                    rhs=pose_p[p][64 : 64 + K1, hs], start=True, stop=True,
                    tile_position=(64, 0),
                )
            epilogue(h1_all[0:M0, c0 * BATCH : (c0 + 1) * BATCH], ps0[0:M0, :], relu=True)
            epilogue(h1_all[0:M1, c1 * BATCH : (c1 + 1) * BATCH], ps1[0:M1, :], relu=True)
            filler(2)

        # L2: plain per-chunk matmuls (K includes the ones/bias row).
        for c, (js, je) in enumerate(CHUNKS):
            nj = je - js
            K, M = 18 * nj + 1, 32 * nj
            h2[c] = work.tile([M, BATCH], f16, tag=f"h2_{c}", name=f"h2_{c}")
            ps = ps_tile(f"ps2_{c}")
            for h, hs in enumerate(HALVES):
                nc.tensor.matmul(
                    ps[0:M, hs], lhsT=w_sb[0:K, W2_OFF[c] : W2_OFF[c] + M],
                    rhs=h1_all[0:K, c * BATCH + hs.start : c * BATCH + hs.stop],
                    start=True, stop=True,
                )
            epilogue(h2[c][:, :], ps[0:M, :], relu=True)
            filler(2)

        # L3: col-tiled into coffT layout; pure-copy epilogues (b3 is folded
        # into the main B-pass via bias_vc). Group B lands twice (cols 0/32
        # and 64/96) for the concurrent B-passes. Processed HALF-MAJOR with
        # per-half epilogues: main b-tiles 0-3 need only the half-0
        # coefficients, so the main GEMM starts ~2us earlier and the half-1
        # coff epilogues hide under it.
        psA = ps_tile("ps3a")
        psB = ps_tile("ps3b")
        for h, hs in enumerate(HALVES):
            for c in range(4):
                nc.tensor.matmul(
                    psA[32 * c : 32 * c + 32, hs],
                    lhsT=w_sb[0:128, W3_OFF[c] : W3_OFF[c] + 32],
                    rhs=h2[c][:, hs], start=True, stop=True,
                    tile_position=(0, 32 * c),
                )
            for r in (0, 64):
                nc.tensor.matmul(
                    psB[r : r + 32, hs], lhsT=w_sb[0:128, W3_OFF[4] : W3_OFF[4] + 32],
                    rhs=h2[4][:, hs], start=True, stop=True, tile_position=(0, r),
                )
                nc.tensor.matmul(
                    psB[r + 32 : r + 56, hs], lhsT=w_sb[0:96, W3_OFF[5] : W3_OFF[5] + 24],
                    rhs=h2[5][:, hs], start=True, stop=True, tile_position=(0, r + 32),
                )
            epilogue(coffT_a[h][:, :], psA[:, hs])
            epilogue(coffT_b[h][0:56, :], psB[0:56, hs])
            epilogue(coffT_b[h][64:120, :], psB[64:120, hs])
            filler(2)
        filler(2)
        # ---- main GEMM, b-tiles in pairs: per 1024-wide N-chunk, A-passes
        # (K=128) for both b-tiles, then the two K=57 B-passes (bias row
        # included) CONCURRENTLY in PE row groups 0 / 64; evacuation with
        # the 2^-13 descale on ACT (tile i) and DVE (tile j) in parallel.
        for p in range(NB // 2):
            bti, btj = 2 * p, 2 * p + 1
            hh = p // 2  # batch half this pair belongs to
            bsl_i = slice(bti * 128, bti * 128 + 128)
            bsl_j = slice(btj * 128, btj * 128 + 128)
            hsl_i = slice(bti % 4 * 128, bti % 4 * 128 + 128)
            hsl_j = slice(btj % 4 * 128, btj % 4 * 128 + 128)
            os_i = outp.tile([128, VC3], f16, tag="ostrip", name=f"ostrip_{bti}")
            os_j = outp.tile([128, VC3], f16, tag="ostrip", name=f"ostrip_{btj}")
            for g in range(3):
                g0, g1 = PAIR_BOUNDS[g], PAIR_BOUNDS[g + 1]
                ti = ps_tile(f"psm_{p}_{g}_i", reserve=False)
                tj = ps_tile(f"psm_{p}_{g}_j", reserve=False)
                subs = [
                    (slice(n0 - g0, n1 - g0), slice(n0, n1))
                    for n0, n1 in zip(NT_BOUNDS, NT_BOUNDS[1:])
                    if g0 <= n0 < g1
                ]
                for ps, hsl in ((ti, hsl_i), (tj, hsl_j)):
                    for ssl, nsl in subs:
                        nc.tensor.matmul(
                            ps[:, ssl], lhsT=coffT_a[hh][:, hsl], rhs=bfm_a[:, nsl],
                            start=True, stop=False,
                        )
                for ssl, nsl in subs:
                    nc.tensor.matmul(
                        ti[:, ssl], lhsT=coffT_b[hh][0:57, hsl_i],
                        rhs=bfm_b[0:57, nsl], start=False, stop=True,
                        tile_position=(0, 0),
                    )
                    nc.tensor.matmul(
                        tj[:, ssl], lhsT=coffT_b[hh][64:121, hsl_j],
                        rhs=bfm_b[64:121, nsl], start=False, stop=True,
                        tile_position=(64, 0),
                    )
                last = p == NB // 2 - 1
                if last and g == 2:
                    # final chunk: evacuate + store 512 then 32 cols so the
                    # very last transfer is tiny (short tail)
                    nc.scalar.activation(
                        os_i[:, 2048:2560], ti[:, 0:512], AF.Copy, scale=DESCALE
                    )
                    nc.vector.tensor_scalar(
                        out=os_j[:, 2048:2560], in0=tj[:, 0:512], scalar1=DESCALE,
                        scalar2=None, op0=ALU.mult,
                    )
                    nc.sync.dma_start(out=res[bsl_i, 2048:2560], in_=os_i[:, 2048:2560])
                    nc.gpsimd.dma_start(out=res[bsl_j, 2048:2560], in_=os_j[:, 2048:2560])
                    nc.scalar.activation(
                        os_i[:, 2560:2592], ti[:, 512:544], AF.Copy, scale=DESCALE
                    )
                    nc.vector.tensor_scalar(
                        out=os_j[:, 2560:2592], in0=tj[:, 512:544], scalar1=DESCALE,
                        scalar2=None, op0=ALU.mult,
                    )
                    nc.sync.dma_start(out=res[bsl_i, 2560:2592], in_=os_i[:, 2560:2592])
                    nc.gpsimd.dma_start(out=res[bsl_j, 2560:2592], in_=os_j[:, 2560:2592])
                    continue
                nc.scalar.activation(
                    os_i[:, g0:g1], ti[:, 0 : g1 - g0], AF.Copy, scale=DESCALE
                )
                nc.vector.tensor_scalar(
                    out=os_j[:, g0:g1], in0=tj[:, 0 : g1 - g0], scalar1=DESCALE,
                    scalar2=None, op0=ALU.mult,
                )
                if last:
                    nc.sync.dma_start(out=res[bsl_i, g0:g1], in_=os_i[:, g0:g1])
                    nc.gpsimd.dma_start(out=res[bsl_j, g0:g1], in_=os_j[:, g0:g1])
            if p < NB // 2 - 1:
                # full-row stores: 5184B HBM segments move faster than the
                # 2048B segments of column-piece stores
                nc.sync.dma_start(out=res[bsl_i, :], in_=os_i[:])
                nc.sync.dma_start(out=res[bsl_j, :], in_=os_j[:])

    nc.finalize()
    return nc


def _pack_host(pose, basis, mask, w1, b1, w2, b2, w3, b3):
    pose_t = pose[:, 1:].reshape(BATCH, 207).T.astype(np.float16)  # [207, B]
    pose_mega = np.zeros((128, 3 * BATCH), np.float16)
    for c, (js, je) in enumerate(CHUNKS):
        K = 9 * (je - js)
        p, hi = divmod(c, 2)
        r0 = 64 if hi else 0
        pose_mega[r0 : r0 + K, p * BATCH : (p + 1) * BATCH] = (
            pose_t[9 * js : 9 * js + K]
        )
        pose_mega[r0 + K, p * BATCH : (p + 1) * BATCH] = 1.0  # ones/bias row

    # bfm rows (j, k) scaled by 2^13, cols (v, c) padded to VPAD.
    bfm = np.zeros((N_JOINT * BPJ, VPAD * 3), np.float32)
    prod = (basis[:, None, :, :] * mask[:, :, None, None] * BSCALE)  # (V, J, K, 3)
    bfm[:, : N_VERT * 3] = prod.transpose(1, 2, 0, 3).reshape(
        N_JOINT * BPJ, N_VERT * 3
    )
    # b3 folded into the main GEMM: bias_vc = b3 . bfm (b-independent).
    bias_vc = (b3.reshape(-1).astype(np.float64) @ bfm.astype(np.float64)).astype(
        np.float32
    )

    w_all = np.zeros((128, W_COLS), np.float16)
    eye9 = np.eye(3, dtype=np.float64).reshape(-1)
    b1f = (
        b1.astype(np.float64) - np.einsum("i,jio->jo", eye9, w1.astype(np.float64))
    ).astype(np.float32)
    for c, ((js, je), o1, o2, o3) in enumerate(zip(CHUNKS, W1_OFF, W2_OFF, W3_OFF)):
        nj = je - js
        r1 = 64 if c % 2 else 0  # odd chunks' W1 blocks live at PE rows 64+
        for t, j in enumerate(range(js, je)):
            w_all[r1 + t * 9 : r1 + (t + 1) * 9, o1 + t * 18 : o1 + (t + 1) * 18] = w1[j]
            w_all[t * 18 : (t + 1) * 18, o2 + t * 32 : o2 + (t + 1) * 32] = w2[j]
            w_all[t * 32 : (t + 1) * 32, o3 + t * 8 : o3 + (t + 1) * 8] = w3[j]
        # bias rows (matched to the activations' ones rows)
        w_all[r1 + 9 * nj, o1 : o1 + 18 * nj] = b1f[js:je].reshape(-1)
        w_all[18 * nj, o2 : o2 + 32 * nj] = b2[js:je].reshape(-1)

    mega = np.concatenate([w_all, pose_mega], axis=1)

    bfm16 = bfm.astype(np.float16)
    bfm_b = np.zeros((128, VPAD * 3), np.float16)
    bfm_b[0:56] = bfm16[128:184]
    bfm_b[56] = bias_vc.astype(np.float16)
    bfm_b[64:120] = bfm16[128:184]
    bfm_b[120] = bias_vc.astype(np.float16)
    return mega, bfm16[0:128], bfm_b


def _in_maps(pose, basis, mask, w1, b1, w2, b2, w3, b3):
    mega, bfm_a, bfm_b = _pack_host(
        np.asarray(pose, np.float32),
        np.asarray(basis, np.float32),
        np.asarray(mask, np.float32),
        np.asarray(w1, np.float32),
        np.asarray(b1, np.float32),
        np.asarray(w2, np.float32),
        np.asarray(b2, np.float32),
        np.asarray(w3, np.float32),
        np.asarray(b3, np.float32),
    )
    ones = np.ones((1, 6 * BATCH), np.float16)
    maps = []
    for i in range(8):
        c0 = i * VC3
        maps.append(
            {
                "mega": mega,
                "bfm_a": np.ascontiguousarray(bfm_a[:, c0 : c0 + VC3]),
                "bfm_b": np.ascontiguousarray(bfm_b[:, c0 : c0 + VC3]),
                "ones": ones,
            }
        )
    return maps


def kernel(pose, basis, mask, w1, b1, w2, b2, w3, b3):
    from concourse.bass_utils import run_bass_kernel_spmd

    if "nc" not in _CACHED:
        _CACHED["nc"] = _build_nc()
    nc = _CACHED["nc"]

    maps = _in_maps(pose, basis, mask, w1, b1, w2, b2, w3, b3)
    r = run_bass_kernel_spmd(nc, maps, core_ids=list(range(8)))
    out = np.concatenate(
        [m["res"].astype(np.float32).reshape(BATCH, VC, 3) for m in r.results],
        axis=1,
    )
    return np.ascontiguousarray(out[:, :N_VERT, :])
